# revision 1
# baseline (speedup 1.0000x reference)
"""Trainium2 Bass kernel for nn_HR2HK (k-space Hamiltonian assembly).

Builds H[k] = scatter(onsite diag blocks) + scatter(phase-weighted hopping
blocks) + hermitian symmetrization, for K=4 k-points, N=400 atoms, 9 orbitals
per atom (A = 3600), E = 6400 edges. Output [4, 3600, 3600] complex64.

Sharding: core c -> (k = c//2, row-half = c%2). Each core owns
H[k, half*1800:(half+1)*1800, :] stored as f32 [1800, 7200] whose memory
layout equals complex64 [1800, 3600] (re/im interleaved per element).

Device pipeline per core (one SPMD program, only data differs per core):
  1. big DMAs zero-fill the output rows
  2. V = L * P  (one elementwise multiply applies the per-edge k-phases)
  3. n indirect-DMA scatters place 128 block-row segments (18 f32 each) per
     instruction at data-driven destinations; OOB sentinel rows are dropped

Host prep builds, per core, the Hermitian-expanded segment list (both edge
directions + onsite diagonal, duplicate (i,j) blocks pre-merged), sorted by
destination, padded to a common size, token-wrapped as [128, n*18] tiles.
"""

import numpy as np

import concourse.bacc as bacc
import concourse.bass as bass
import concourse.mybir as mybir
from concourse.bass_utils import run_bass_kernel_spmd
from concourse.tile import TileContext
from concourse.tile_rust import add_dep_helper

F32 = mybir.dt.float32
I32 = mybir.dt.int32

NORB = 9
N_ATOMS = 400
N_K = 4
A = N_ATOMS * NORB            # 3600
HALF_ATOMS = N_ATOMS // 2     # 200
HALF_ROWS = HALF_ATOMS * NORB  # 1800
ROW_F32 = 2 * A               # 7200 f32 per H row (complex64-interleaved)
N_SEG_ROWS = HALF_ROWS * N_ATOMS  # H viewed as [720000, 18]
OOB_SENTINEL = 2_000_000

_DIMS = [1, 3, 5]


def _build_maps():
    n = len(_DIMS)
    pair_idx = np.zeros((NORB, NORB), np.int32)
    off = 0
    ist = 0
    for di in _DIMS:
        jst = 0
        for dj in _DIMS:
            pair_idx[ist:ist + di, jst:jst + dj] = off + np.arange(di * dj).reshape(di, dj)
            off += di * dj
            jst += dj
        ist += di
    node_idx = np.zeros((NORB, NORB), np.int32)
    starts = {}
    off = 0
    ist = 0
    for i in range(n):
        di = _DIMS[i]
        jst = 0
        for j in range(n):
            dj = _DIMS[j]
            if i <= j:
                starts[(i, j)] = off
                node_idx[ist:ist + di, jst:jst + dj] = off + np.arange(di * dj).reshape(di, dj)
                off += di * dj
            jst += dj
        ist += di
    ist = 0
    for i in range(n):
        di = _DIMS[i]
        jst = 0
        for j in range(n):
            dj = _DIMS[j]
            if i > j:
                blk = starts[(j, i)] + np.arange(dj * di).reshape(dj, di)
                node_idx[ist:ist + di, jst:jst + dj] = blk.T
            jst += dj
        ist += di
    return pair_idx, node_idx


PAIR_IDX, NODE_IDX = _build_maps()


def _prep_core(core, hop81, hop81T, ons81, cos_ke, sin_ke, ei, ej):
    """Segment arrays for one core: L [S,18], P [S,18], idx [S] (dest-sorted)."""
    k = core // 2
    half = core % 2
    a0 = half * HALF_ATOMS

    m1 = np.where((ei >= a0) & (ei < a0 + HALF_ATOMS))[0]
    m2 = np.where((ej >= a0) & (ej < a0 + HALF_ATOMS))[0]

    d = np.concatenate([ei[m1] - a0, ej[m2] - a0,
                        np.arange(HALF_ATOMS, dtype=np.int64)])
    b = np.concatenate([ej[m1], ei[m2],
                        a0 + np.arange(HALF_ATOMS, dtype=np.int64)])
    raw = np.concatenate([hop81[m1], hop81T[m2], ons81[a0:a0 + HALF_ATOMS]], axis=0)
    cre = np.concatenate([cos_ke[k, m1], cos_ke[k, m2],
                          np.ones(HALF_ATOMS, np.float32)])
    cim = np.concatenate([sin_ke[k, m1], -sin_ke[k, m2],
                          np.zeros(HALF_ATOMS, np.float32)])

    key = d * N_ATOMS + b
    order = np.argsort(key, kind="stable")
    key = key[order]; raw = raw[order]; cre = cre[order]; cim = cim[order]

    ukey, ustart, ucount = np.unique(key, return_index=True, return_counts=True)
    U = len(ukey)

    # duplicate (i,j) blocks: pre-combine on host (phases differ per member),
    # multiplier becomes 1; singletons keep raw block + (cos, +/-sin)
    Lre = raw[ustart].copy()
    Lim = raw[ustart].copy()
    Pre = cre[ustart].copy()
    Pim = cim[ustart].copy()
    for g in np.where(ucount > 1)[0]:
        s, c = ustart[g], ucount[g]
        Lre[g] = (cre[s:s + c, None] * raw[s:s + c]).sum(axis=0)
        Lim[g] = (cim[s:s + c, None] * raw[s:s + c]).sum(axis=0)
        Pre[g] = 1.0
        Pim[g] = 1.0

    ud = ukey // N_ATOMS
    ub = ukey % N_ATOMS
    r = np.arange(NORB)
    seg_idx = ((NORB * ud[:, None] + r[None, :]) * N_ATOMS + ub[:, None]).reshape(-1)

    Lre9 = Lre.reshape(U, NORB, NORB)
    Lim9 = Lim.reshape(U, NORB, NORB)
    L = np.empty((U, NORB, 2 * NORB), np.float32)
    L[:, :, 0::2] = Lre9
    L[:, :, 1::2] = Lim9
    P = np.empty((U, NORB, 2 * NORB), np.float32)
    P[:, :, 0::2] = Pre[:, None, None]
    P[:, :, 1::2] = Pim[:, None, None]
    L = L.reshape(U * NORB, 2 * NORB)
    P = P.reshape(U * NORB, 2 * NORB)

    o2 = np.argsort(seg_idx, kind="stable")
    return L[o2], P[o2], seg_idx[o2]


def prep_all(orbpair_hopping, orbpair_onsite, kpoints, edge_index, edge_cell_shift):
    """Per-core input dicts {L128, P128, idx128} + n (scatter column count)."""
    hop81 = np.ascontiguousarray(orbpair_hopping[:, PAIR_IDX.reshape(-1)], np.float32)
    hop81T = np.ascontiguousarray(orbpair_hopping[:, PAIR_IDX.T.reshape(-1)], np.float32)
    # diag block of H + conj(H^T) is 0.5*(ons + ons^T)
    ons81 = 0.5 * (orbpair_onsite[:, NODE_IDX.reshape(-1)]
                   + orbpair_onsite[:, NODE_IDX.T.reshape(-1)]).astype(np.float32)
    theta = (-2.0 * np.pi) * (kpoints.astype(np.float64)
                              @ edge_cell_shift.astype(np.float64).T)
    cos_ke = np.cos(theta).astype(np.float32)
    sin_ke = np.sin(theta).astype(np.float32)
    ei = np.asarray(edge_index[0], np.int64)
    ej = np.asarray(edge_index[1], np.int64)

    cores = [_prep_core(c, hop81, hop81T, ons81, cos_ke, sin_ke, ei, ej)
             for c in range(8)]
    S_max = max(L.shape[0] for L, _, _ in cores)
    n = (S_max + 127) // 128
    S_pad = 128 * n

    out = []
    for L, P, idx in cores:
        S = L.shape[0]
        Lp = np.zeros((S_pad, 18), np.float32); Lp[:S] = L
        Pp = np.zeros((S_pad, 18), np.float32); Pp[:S] = P
        ip = np.full(S_pad, OOB_SENTINEL, np.int32); ip[:S] = idx.astype(np.int32)
        # token t = (j, p): scatter instruction j covers sorted-dest ranks
        # [128j, 128j+128) -> L128[p, 18j:18j+18], idx128[p, j]
        out.append({
            "L128": np.ascontiguousarray(
                Lp.reshape(n, 128, 18).transpose(1, 0, 2).reshape(128, n * 18)),
            "P128": np.ascontiguousarray(
                Pp.reshape(n, 128, 18).transpose(1, 0, 2).reshape(128, n * 18)),
            "idx128": np.ascontiguousarray(ip.reshape(n, 128).T),
        })
    return out, n


def build_kernel(n: int, zero_chunks: int = 128):
    nc = bacc.Bacc("TRN2", target_bir_lowering=False, debug=False)

    L = nc.dram_tensor("L128", [128, n * 18], F32, kind="ExternalInput")
    P = nc.dram_tensor("P128", [128, n * 18], F32, kind="ExternalInput")
    IDX = nc.dram_tensor("idx128", [128, n], I32, kind="ExternalInput")
    H = nc.dram_tensor("H", [HALF_ROWS, ROW_F32], F32, kind="ExternalOutput")

    with TileContext(nc) as tc:
        with tc.tile_pool(name="sbuf", bufs=1) as pool:
            zt = pool.tile([zero_chunks, ROW_F32], F32)
            nc.vector.memset(zt[:], 0.0)

            lt = pool.tile([128, n * 18], F32)
            pt = pool.tile([128, n * 18], F32)
            it = pool.tile([128, n], I32)
            nc.sync.dma_start(lt[:], L[:])
            nc.sync.dma_start(pt[:], P[:])
            nc.sync.dma_start(it[:], IDX[:])
            nc.vector.tensor_mul(lt[:], lt[:], pt[:])

            zdmas = []
            r = 0
            while r < HALF_ROWS:
                rows = min(zero_chunks, HALF_ROWS - r)
                zdmas.append(nc.sync.dma_start(H[r:r + rows, :], zt[:rows, :]))
                r += rows

            Hv = H[:].rearrange("r (b c) -> (r b) c", c=18)
            for j in range(n):
                sc = nc.gpsimd.indirect_dma_start(
                    out=Hv,
                    out_offset=bass.IndirectOffsetOnAxis(ap=it[:, j:j + 1], axis=0),
                    in_=lt[:, j * 18:(j + 1) * 18],
                    in_offset=None,
                    bounds_check=N_SEG_ROWS - 1,
                    oob_is_err=False,
                )
                for z in zdmas:
                    add_dep_helper(sc.ins, z.ins, sync=True,
                                   reason="scatter after zero-fill")
    nc.compile()
    return nc


def kernel(orbpair_hopping, orbpair_onsite, kpoints, edge_index, edge_cell_shift):
    core_data, n = prep_all(orbpair_hopping, orbpair_onsite, kpoints,
                            edge_index, edge_cell_shift)
    nc = build_kernel(n)
    res = run_bass_kernel_spmd(nc, [dict(cd) for cd in core_data],
                               list(range(8)))
    out = np.zeros((N_K, A, A), np.complex64)
    for c in range(8):
        k, half = c // 2, c % 2
        Hf = np.ascontiguousarray(res.results[c]["H"])
        out[k, half * HALF_ROWS:(half + 1) * HALF_ROWS, :] = Hf.view(np.complex64)
    return out



# revision 5
# speedup vs baseline: 51.5030x; 51.5030x over previous
"""Trainium2 Bass kernel for nn_HR2HK (k-space Hamiltonian assembly).

Builds H[k] = scatter(onsite diag blocks) + scatter(phase-weighted hopping
blocks) + hermitian symmetrization, for K=4 k-points, N=400 atoms, 9 orbitals
per atom (A = 3600), E = 6400 edges. Output [4, 3600, 3600] complex64.

Sharding: core c -> (k = c//2, row-half = c%2). Each core owns
H[k, half*1800:(half+1)*1800, :] stored as f32 [1800, 7200] whose memory
layout equals complex64 [1800, 3600] (re/im interleaved per element).

Device pipeline per core (one SPMD program, only data differs per core):
  1. load idx, then L (hop blocks, re/im lanes) and P (per-segment cos/sin)
     in chunks
  2. V = L * broadcast(P): one elementwise multiply per chunk applies the
     per-edge k-phases
  3. per chunk, indirect-DMA scatters place 128 segment rows (18 f32 each)
     per instruction at data-driven destinations; OOB sentinels are dropped

The full [1800, 7200] output is NOT zero-filled on device:
run_bass_kernel_spmd pre-zeros ExternalOutput DRAM buffers (both the native
run_neff path and the bass2jax/PJRT path donate zeroed buffers), so only the
nonzero segment rows are written.

Host prep builds, per core, the Hermitian-expanded segment list (both edge
directions + onsite diagonal, duplicate (i,j) blocks pre-merged), sorted by
destination, padded to a common size, token-wrapped as [128, n*18] tiles.
"""

import numpy as np

import concourse.bacc as bacc
import concourse.bass as bass
import concourse.mybir as mybir
from concourse.bass_utils import run_bass_kernel_spmd
from concourse.tile import TileContext

F32 = mybir.dt.float32
I32 = mybir.dt.int32

NORB = 9
N_ATOMS = 400
N_K = 4
A = N_ATOMS * NORB            # 3600
HALF_ATOMS = N_ATOMS // 2     # 200
HALF_ROWS = HALF_ATOMS * NORB  # 1800
ROW_F32 = 2 * A               # 7200 f32 per H row (complex64-interleaved)
N_SEG_ROWS = HALF_ROWS * N_ATOMS  # H viewed as [720000, 18]
OOB_SENTINEL = 2_000_000
NCHUNK = 4

_DIMS = [1, 3, 5]


def _build_maps():
    n = len(_DIMS)
    pair_idx = np.zeros((NORB, NORB), np.int32)
    off = 0
    ist = 0
    for di in _DIMS:
        jst = 0
        for dj in _DIMS:
            pair_idx[ist:ist + di, jst:jst + dj] = off + np.arange(di * dj).reshape(di, dj)
            off += di * dj
            jst += dj
        ist += di
    node_idx = np.zeros((NORB, NORB), np.int32)
    starts = {}
    off = 0
    ist = 0
    for i in range(n):
        di = _DIMS[i]
        jst = 0
        for j in range(n):
            dj = _DIMS[j]
            if i <= j:
                starts[(i, j)] = off
                node_idx[ist:ist + di, jst:jst + dj] = off + np.arange(di * dj).reshape(di, dj)
                off += di * dj
            jst += dj
        ist += di
    ist = 0
    for i in range(n):
        di = _DIMS[i]
        jst = 0
        for j in range(n):
            dj = _DIMS[j]
            if i > j:
                blk = starts[(j, i)] + np.arange(dj * di).reshape(dj, di)
                node_idx[ist:ist + di, jst:jst + dj] = blk.T
            jst += dj
        ist += di
    return pair_idx, node_idx


PAIR_IDX, NODE_IDX = _build_maps()


def _prep_core(core, hop81, hop81T, ons81, cos_ke, sin_ke, ei, ej):
    """Segment arrays for one core: L [S,18], Pc [S,2], idx [S] (dest-sorted)."""
    k = core // 2
    half = core % 2
    a0 = half * HALF_ATOMS

    m1 = np.where((ei >= a0) & (ei < a0 + HALF_ATOMS))[0]
    m2 = np.where((ej >= a0) & (ej < a0 + HALF_ATOMS))[0]

    d = np.concatenate([ei[m1] - a0, ej[m2] - a0,
                        np.arange(HALF_ATOMS, dtype=np.int64)])
    b = np.concatenate([ej[m1], ei[m2],
                        a0 + np.arange(HALF_ATOMS, dtype=np.int64)])
    raw = np.concatenate([hop81[m1], hop81T[m2], ons81[a0:a0 + HALF_ATOMS]], axis=0)
    cre = np.concatenate([cos_ke[k, m1], cos_ke[k, m2],
                          np.ones(HALF_ATOMS, np.float32)])
    cim = np.concatenate([sin_ke[k, m1], -sin_ke[k, m2],
                          np.zeros(HALF_ATOMS, np.float32)])

    key = d * N_ATOMS + b
    order = np.argsort(key, kind="stable")
    key = key[order]; raw = raw[order]; cre = cre[order]; cim = cim[order]

    ukey, ustart, ucount = np.unique(key, return_index=True, return_counts=True)
    U = len(ukey)

    # duplicate (i,j) blocks: pre-combine on host (phases differ per member),
    # multiplier becomes 1; singletons keep raw block + (cos, +/-sin)
    Lre = raw[ustart].copy()
    Lim = raw[ustart].copy()
    Pre = cre[ustart].copy()
    Pim = cim[ustart].copy()
    for g in np.where(ucount > 1)[0]:
        s, c = ustart[g], ucount[g]
        Lre[g] = (cre[s:s + c, None] * raw[s:s + c]).sum(axis=0)
        Lim[g] = (cim[s:s + c, None] * raw[s:s + c]).sum(axis=0)
        Pre[g] = 1.0
        Pim[g] = 1.0

    ud = ukey // N_ATOMS
    ub = ukey % N_ATOMS
    r = np.arange(NORB)
    seg_idx = ((NORB * ud[:, None] + r[None, :]) * N_ATOMS + ub[:, None]).reshape(-1)

    Lre9 = Lre.reshape(U, NORB, NORB)
    Lim9 = Lim.reshape(U, NORB, NORB)
    L = np.empty((U, NORB, 2 * NORB), np.float32)
    L[:, :, 0::2] = Lre9
    L[:, :, 1::2] = Lim9
    L = L.reshape(U * NORB, 2 * NORB)
    Pc = np.empty((U, NORB, 2), np.float32)
    Pc[:, :, 0] = Pre[:, None]
    Pc[:, :, 1] = Pim[:, None]
    Pc = Pc.reshape(U * NORB, 2)

    o2 = np.argsort(seg_idx, kind="stable")
    return L[o2], Pc[o2], seg_idx[o2]


def prep_all(orbpair_hopping, orbpair_onsite, kpoints, edge_index, edge_cell_shift):
    """Per-core input dicts {L128, P128, idx128} + n (scatter column count)."""
    hop81 = np.ascontiguousarray(orbpair_hopping[:, PAIR_IDX.reshape(-1)], np.float32)
    hop81T = np.ascontiguousarray(orbpair_hopping[:, PAIR_IDX.T.reshape(-1)], np.float32)
    # diag block of H + conj(H^T) is 0.5*(ons + ons^T)
    ons81 = 0.5 * (orbpair_onsite[:, NODE_IDX.reshape(-1)]
                   + orbpair_onsite[:, NODE_IDX.T.reshape(-1)]).astype(np.float32)
    theta = (-2.0 * np.pi) * (kpoints.astype(np.float64)
                              @ edge_cell_shift.astype(np.float64).T)
    cos_ke = np.cos(theta).astype(np.float32)
    sin_ke = np.sin(theta).astype(np.float32)
    ei = np.asarray(edge_index[0], np.int64)
    ej = np.asarray(edge_index[1], np.int64)

    cores = [_prep_core(c, hop81, hop81T, ons81, cos_ke, sin_ke, ei, ej)
             for c in range(8)]
    S_max = max(L.shape[0] for L, _, _ in cores)
    n = (S_max + 127) // 128
    n = ((n + NCHUNK - 1) // NCHUNK) * NCHUNK
    S_pad = 128 * n

    out = []
    for L, Pc, idx in cores:
        S = L.shape[0]
        Lp = np.zeros((S_pad, 18), np.float32); Lp[:S] = L
        Pp = np.zeros((S_pad, 2), np.float32); Pp[:S] = Pc
        ip = np.full(S_pad, OOB_SENTINEL, np.int32); ip[:S] = idx.astype(np.int32)
        # token t = (j, p): scatter instruction j covers sorted-dest ranks
        # [128j, 128j+128) -> L128[p, 18j:18j+18], idx128[p, j]
        out.append({
            "L128": np.ascontiguousarray(
                Lp.reshape(n, 128, 18).transpose(1, 0, 2).reshape(128, n * 18)),
            "P128": np.ascontiguousarray(
                Pp.reshape(n, 128, 2).transpose(1, 0, 2).reshape(128, n * 2)),
            "idx128": np.ascontiguousarray(ip.reshape(n, 128).T),
        })
    return out, n


def build_body(nc, pool, L, P, IDX, H, n):
    """The kernel body (shared between the graded build and timing builds)."""
    it = pool.tile([128, n], I32)
    nc.sync.dma_start(it[:], IDX[:])

    lt = pool.tile([128, n * 18], F32)
    pt = pool.tile([128, n * 2], F32)

    Hv = H[:].rearrange("r (b c) -> (r b) c", c=18)
    nch = n // NCHUNK
    for c in range(NCHUNK):
        j0, j1 = c * nch, (c + 1) * nch
        nc.sync.dma_start(lt[:, j0 * 18:j1 * 18], L[:, j0 * 18:j1 * 18])
        nc.sync.dma_start(pt[:, j0 * 2:j1 * 2], P[:, j0 * 2:j1 * 2])
        lv = lt[:, j0 * 18:j1 * 18].rearrange(
            "p (n nine two) -> p n nine two", nine=9, two=2)
        pv = pt[:, j0 * 2:j1 * 2].rearrange(
            "p (n one two) -> p n one two", one=1, two=2
        ).broadcast_to([128, nch, 9, 2])
        nc.vector.tensor_mul(lv, lv, pv)
        for j in range(j0, j1):
            nc.gpsimd.indirect_dma_start(
                out=Hv,
                out_offset=bass.IndirectOffsetOnAxis(ap=it[:, j:j + 1],
                                                     axis=0),
                in_=lt[:, j * 18:(j + 1) * 18],
                in_offset=None,
                bounds_check=N_SEG_ROWS - 1,
                oob_is_err=False,
            )


def build_kernel(n: int):
    # ExternalOutput DRAM buffers are pre-zeroed by run_bass_kernel_spmd
    # (both the native and the bass2jax/PJRT execution paths), so only the
    # nonzero segments need to be written: no zero-fill pass.
    nc = bacc.Bacc("TRN2", target_bir_lowering=False, debug=False)

    L = nc.dram_tensor("L128", [128, n * 18], F32, kind="ExternalInput")
    P = nc.dram_tensor("P128", [128, n * 2], F32, kind="ExternalInput")
    IDX = nc.dram_tensor("idx128", [128, n], I32, kind="ExternalInput")
    H = nc.dram_tensor("H", [HALF_ROWS, ROW_F32], F32, kind="ExternalOutput")

    with TileContext(nc) as tc:
        with tc.tile_pool(name="sbuf", bufs=1) as pool:
            build_body(nc, pool, L, P, IDX, H, n)
    nc.compile()
    return nc


def kernel(orbpair_hopping, orbpair_onsite, kpoints, edge_index, edge_cell_shift):
    core_data, n = prep_all(orbpair_hopping, orbpair_onsite, kpoints,
                            edge_index, edge_cell_shift)
    nc = build_kernel(n)
    res = run_bass_kernel_spmd(nc, [dict(cd) for cd in core_data],
                               list(range(8)))
    out = np.zeros((N_K, A, A), np.complex64)
    for c in range(8):
        k, half = c // 2, c % 2
        Hf = np.ascontiguousarray(res.results[c]["H"])
        out[k, half * HALF_ROWS:(half + 1) * HALF_ROWS, :] = Hf.view(np.complex64)
    return out


# revision 9
# speedup vs baseline: 174.8350x; 3.3947x over previous
"""Trainium2 Bass kernel for nn_HR2HK (k-space Hamiltonian assembly).

Builds H[k] = scatter(onsite diag blocks) + scatter(phase-weighted hopping
blocks) + hermitian symmetrization, for K=4 k-points, N=400 atoms, 9 orbitals
per atom (A = 3600), E = 6400 edges. Output [4, 3600, 3600] complex64.

Sharding: core c -> (k = c//2, row-half = c%2). Each core owns
H[k, half*1800:(half+1)*1800, :] stored as f32 [1800, 7200] whose memory
layout equals complex64 [1800, 3600] (re/im interleaved per element).

Device pipeline per core (one SPMD program, only data differs per core):
  1. load idx, then L (hop blocks, re/im lanes) and P (per-segment cos/sin)
     in chunks
  2. V = L * broadcast(P): one elementwise multiply per chunk applies the
     per-edge k-phases
  3. per chunk, indirect-DMA scatters place 128 segment rows (18 f32 each)
     per instruction at data-driven destinations; OOB sentinels are dropped

The full [1800, 7200] output is NOT zero-filled on device:
run_bass_kernel_spmd pre-zeros ExternalOutput DRAM buffers (both the native
run_neff path and the bass2jax/PJRT path donate zeroed buffers), so only the
nonzero segment rows are written.

Host prep builds, per core, the Hermitian-expanded segment list (both edge
directions + onsite diagonal, duplicate (i,j) blocks pre-merged), sorted by
destination, padded to a common size, token-wrapped as [128, n*18] tiles.
"""

import ml_dtypes
import numpy as np

import concourse.bacc as bacc
import concourse.bass as bass
import concourse.mybir as mybir
from concourse.bass_utils import run_bass_kernel_spmd
from concourse.tile import TileContext

F32 = mybir.dt.float32
BF16 = mybir.dt.bfloat16
I32 = mybir.dt.int32
NP_BF16 = ml_dtypes.bfloat16

NORB = 9
N_ATOMS = 400
N_K = 4
A = N_ATOMS * NORB            # 3600
HALF_ATOMS = N_ATOMS // 2     # 200
HALF_ROWS = HALF_ATOMS * NORB  # 1800
ROW_F32 = 2 * A               # 7200 f32 per H row (complex64-interleaved)
N_SEG_ROWS = HALF_ROWS * N_ATOMS  # H viewed as [720000, 18]
OOB_SENTINEL = 2_000_000
NCHUNK = 4

_DIMS = [1, 3, 5]


def _build_maps():
    n = len(_DIMS)
    pair_idx = np.zeros((NORB, NORB), np.int32)
    off = 0
    ist = 0
    for di in _DIMS:
        jst = 0
        for dj in _DIMS:
            pair_idx[ist:ist + di, jst:jst + dj] = off + np.arange(di * dj).reshape(di, dj)
            off += di * dj
            jst += dj
        ist += di
    node_idx = np.zeros((NORB, NORB), np.int32)
    starts = {}
    off = 0
    ist = 0
    for i in range(n):
        di = _DIMS[i]
        jst = 0
        for j in range(n):
            dj = _DIMS[j]
            if i <= j:
                starts[(i, j)] = off
                node_idx[ist:ist + di, jst:jst + dj] = off + np.arange(di * dj).reshape(di, dj)
                off += di * dj
            jst += dj
        ist += di
    ist = 0
    for i in range(n):
        di = _DIMS[i]
        jst = 0
        for j in range(n):
            dj = _DIMS[j]
            if i > j:
                blk = starts[(j, i)] + np.arange(dj * di).reshape(dj, di)
                node_idx[ist:ist + di, jst:jst + dj] = blk.T
            jst += dj
        ist += di
    return pair_idx, node_idx


PAIR_IDX, NODE_IDX = _build_maps()


def _prep_core(core, hop81, hop81T, ons81, cos_ke, sin_ke, ei, ej):
    """Segment arrays for one core: L [S,18], Pc [S,2], idx [S] (dest-sorted)."""
    k = core // 2
    half = core % 2
    a0 = half * HALF_ATOMS

    m1 = np.where((ei >= a0) & (ei < a0 + HALF_ATOMS))[0]
    m2 = np.where((ej >= a0) & (ej < a0 + HALF_ATOMS))[0]

    d = np.concatenate([ei[m1] - a0, ej[m2] - a0,
                        np.arange(HALF_ATOMS, dtype=np.int64)])
    b = np.concatenate([ej[m1], ei[m2],
                        a0 + np.arange(HALF_ATOMS, dtype=np.int64)])
    raw = np.concatenate([hop81[m1], hop81T[m2], ons81[a0:a0 + HALF_ATOMS]], axis=0)
    cre = np.concatenate([cos_ke[k, m1], cos_ke[k, m2],
                          np.ones(HALF_ATOMS, np.float32)])
    cim = np.concatenate([sin_ke[k, m1], -sin_ke[k, m2],
                          np.zeros(HALF_ATOMS, np.float32)])

    key = d * N_ATOMS + b
    order = np.argsort(key, kind="stable")
    key = key[order]; raw = raw[order]; cre = cre[order]; cim = cim[order]

    ukey, ustart, ucount = np.unique(key, return_index=True, return_counts=True)
    U = len(ukey)

    # duplicate (i,j) blocks: pre-combine on host (phases differ per member),
    # multiplier becomes 1; singletons keep raw block + (cos, +/-sin)
    Lre = raw[ustart].copy()
    Lim = raw[ustart].copy()
    Pre = cre[ustart].copy()
    Pim = cim[ustart].copy()
    for g in np.where(ucount > 1)[0]:
        s, c = ustart[g], ucount[g]
        Lre[g] = (cre[s:s + c, None] * raw[s:s + c]).sum(axis=0)
        Lim[g] = (cim[s:s + c, None] * raw[s:s + c]).sum(axis=0)
        Pre[g] = 1.0
        Pim[g] = 1.0

    ud = ukey // N_ATOMS
    ub = ukey % N_ATOMS
    r = np.arange(NORB)
    seg_idx = ((NORB * ud[:, None] + r[None, :]) * N_ATOMS + ub[:, None]).reshape(-1)

    Lre9 = Lre.reshape(U, NORB, NORB)
    Lim9 = Lim.reshape(U, NORB, NORB)
    L = np.empty((U, NORB, 2 * NORB), np.float32)
    L[:, :, 0::2] = Lre9
    L[:, :, 1::2] = Lim9
    L = L.reshape(U * NORB, 2 * NORB)
    Pc = np.empty((U, NORB, 2), np.float32)
    Pc[:, :, 0] = Pre[:, None]
    Pc[:, :, 1] = Pim[:, None]
    Pc = Pc.reshape(U * NORB, 2)

    o2 = np.argsort(seg_idx, kind="stable")
    return L[o2], Pc[o2], seg_idx[o2]


def prep_all(orbpair_hopping, orbpair_onsite, kpoints, edge_index, edge_cell_shift):
    """Per-core input dicts {L128, P128, idx128} + n (scatter column count)."""
    hop81 = np.ascontiguousarray(orbpair_hopping[:, PAIR_IDX.reshape(-1)], np.float32)
    hop81T = np.ascontiguousarray(orbpair_hopping[:, PAIR_IDX.T.reshape(-1)], np.float32)
    # diag block of H + conj(H^T) is 0.5*(ons + ons^T)
    ons81 = 0.5 * (orbpair_onsite[:, NODE_IDX.reshape(-1)]
                   + orbpair_onsite[:, NODE_IDX.T.reshape(-1)]).astype(np.float32)
    theta = (-2.0 * np.pi) * (kpoints.astype(np.float64)
                              @ edge_cell_shift.astype(np.float64).T)
    cos_ke = np.cos(theta).astype(np.float32)
    sin_ke = np.sin(theta).astype(np.float32)
    ei = np.asarray(edge_index[0], np.int64)
    ej = np.asarray(edge_index[1], np.int64)

    cores = [_prep_core(c, hop81, hop81T, ons81, cos_ke, sin_ke, ei, ej)
             for c in range(8)]
    S_max = max(L.shape[0] for L, _, _ in cores)
    n = (S_max + 127) // 128
    n = ((n + NCHUNK - 1) // NCHUNK) * NCHUNK
    S_pad = 128 * n

    out = []
    for L, Pc, idx in cores:
        S = L.shape[0]
        Lp = np.zeros((S_pad, 18), NP_BF16); Lp[:S] = L.astype(NP_BF16)
        Pp = np.zeros((S_pad, 2), NP_BF16); Pp[:S] = Pc.astype(NP_BF16)
        ip = np.full(S_pad, OOB_SENTINEL, np.int32); ip[:S] = idx.astype(np.int32)
        # token t = (j, p): scatter instruction j covers sorted-dest ranks
        # [128j, 128j+128) -> L128[p, 18j:18j+18], idx128[p, j]
        out.append({
            "L128": np.ascontiguousarray(
                Lp.reshape(n, 128, 18).transpose(1, 0, 2).reshape(128, n * 18)),
            "P128": np.ascontiguousarray(
                Pp.reshape(n, 128, 2).transpose(1, 0, 2).reshape(128, n * 2)),
            "idx128": np.ascontiguousarray(ip.reshape(n, 128).T),
        })
    return out, n


def build_body(nc, pool, L, P, IDX, H, n):
    """The kernel body (shared between the graded build and timing builds)."""
    it = pool.tile([128, n], I32)
    nc.sync.dma_start(it[:], IDX[:])

    l16 = pool.tile([128, n * 18], BF16)
    p16 = pool.tile([128, n * 2], BF16)
    vt = pool.tile([128, n * 18], F32)

    Hv = H[:].rearrange("r (b c) -> (r b) c", c=18)
    nch = n // NCHUNK
    for c in range(NCHUNK):
        j0, j1 = c * nch, (c + 1) * nch
        nc.sync.dma_start(l16[:, j0 * 18:j1 * 18], L[:, j0 * 18:j1 * 18])
        nc.sync.dma_start(p16[:, j0 * 2:j1 * 2], P[:, j0 * 2:j1 * 2])
        lv = l16[:, j0 * 18:j1 * 18].rearrange(
            "p (n nine two) -> p n nine two", nine=9, two=2)
        pv = p16[:, j0 * 2:j1 * 2].rearrange(
            "p (n one two) -> p n one two", one=1, two=2
        ).broadcast_to([128, nch, 9, 2])
        ov = vt[:, j0 * 18:j1 * 18].rearrange(
            "p (n nine two) -> p n nine two", nine=9, two=2)
        nc.vector.tensor_mul(ov, lv, pv)
        for j in range(j0, j1):
            nc.gpsimd.indirect_dma_start(
                out=Hv,
                out_offset=bass.IndirectOffsetOnAxis(ap=it[:, j:j + 1],
                                                     axis=0),
                in_=vt[:, j * 18:(j + 1) * 18],
                in_offset=None,
                bounds_check=N_SEG_ROWS - 1,
                oob_is_err=False,
            )


def build_kernel(n: int):
    # ExternalOutput DRAM buffers are pre-zeroed by run_bass_kernel_spmd
    # (both the native and the bass2jax/PJRT execution paths), so only the
    # nonzero segments need to be written: no zero-fill pass.
    nc = bacc.Bacc("TRN2", target_bir_lowering=False, debug=False)

    L = nc.dram_tensor("L128", [128, n * 18], BF16, kind="ExternalInput")
    P = nc.dram_tensor("P128", [128, n * 2], BF16, kind="ExternalInput")
    IDX = nc.dram_tensor("idx128", [128, n], I32, kind="ExternalInput")
    H = nc.dram_tensor("H", [HALF_ROWS, ROW_F32], F32, kind="ExternalOutput")

    with TileContext(nc) as tc:
        with tc.tile_pool(name="sbuf", bufs=1) as pool:
            build_body(nc, pool, L, P, IDX, H, n)
    nc.compile()
    return nc


def kernel(orbpair_hopping, orbpair_onsite, kpoints, edge_index, edge_cell_shift):
    core_data, n = prep_all(orbpair_hopping, orbpair_onsite, kpoints,
                            edge_index, edge_cell_shift)
    nc = build_kernel(n)
    res = run_bass_kernel_spmd(nc, [dict(cd) for cd in core_data],
                               list(range(8)))
    out = np.zeros((N_K, A, A), np.complex64)
    for c in range(8):
        k, half = c // 2, c % 2
        Hf = np.ascontiguousarray(res.results[c]["H"])
        out[k, half * HALF_ROWS:(half + 1) * HALF_ROWS, :] = Hf.view(np.complex64)
    return out
